# revision 22
# baseline (speedup 1.0000x reference)
"""De-stationary attention (B=4, L=S=2048, D=512, H=8, dk=64) on 8 TRN2 cores.

Sharding: core c -> batch b = c//2, query-half = c%2 (1024 rows each).
Each core computes full attention for its (batch, q-half) over all 8 heads
using the whole K/V of that batch; outputs concatenate with no reduction.

Math (per batch):
  q = queries @ Wq + bq ; k = keys @ Wk ; v = values @ Wv
  scores = tau * (q . k) / 8 + delta[s]
  attn   = softmax_s(scores)            (no max-subtraction; |scores| <~ 10)
  out    = (attn @ v) @ Wo + bo2        with bo2 = bv @ Wo + bo (host-folded;
           exact since attn rows sum to 1), and bk dropped entirely (a
           per-query constant shift of scores is softmax-invariant).

Device-side structure:
  exp(tau*qk/8 + delta_s) = exp(tau/8 * qk) * w_s with w_s = exp(delta_s)
  folded into V: the AV matmul uses lhsT = [w*v | w] so row 64 of the
  (transposed) AV output accumulates the softmax denominator.
  Layouts are transposed end-to-end (host supplies X^T inputs) so no
  on-device transposes are needed; the final output is natural [q, d].
  Output projection stacks head pairs into K=128 matmuls. The softmax
  reciprocal is broadcast across partitions with gpsimd.partition_broadcast
  (no DRAM round trips). Projection passes are interleaved into the
  attention s-loop so the PE fills its exp-wait gaps and ScalarE (the
  bottleneck engine) never starves.
"""

import os
from contextlib import ExitStack

import numpy as np

import concourse.bass as bass
import concourse.bacc as bacc
import concourse.mybir as mybir
import concourse.tile as tile
from concourse.bass_utils import run_bass_kernel_spmd

# Problem constants (hardcoded per the harness contract).
B, LFULL, S, D = 4, 2048, 2048, 512
H, DK = 8, 64
NCORES = 8
LC = B * LFULL // NCORES  # 1024 query rows per core
NQT = LC // 512           # q-tiles of 512
SC = S // 128             # 16 s-chunks
F32 = mybir.dt.float32
F32R = mybir.dt.float32r
BF16 = mybir.dt.bfloat16

# Matmul dtype knob: "f16" (default: full-rate, 10-bit mantissa — same
# precision class as f32r/tf32 but half the DMA/SBUF bytes), "f32r", "f32".
MM_DTYPE = os.environ.get("KERNEL_MM_DTYPE", "f16")
MDT = {"f16": mybir.dt.float16, "f32r": F32R, "f32": F32}[MM_DTYPE]
NPDT = {"f16": np.float16, "f32r": np.float32, "f32": np.float32}[MM_DTYPE]
# fp16 attention-core operands (kT/qT/vw/P) — full matmul rate with 8x
# finer rounding than bf16. exp values <= ~1.5e4 fit fp16 range.
ADT = MDT if os.environ.get("KERNEL_F16") == "0" else mybir.dt.float16
AF = mybir.ActivationFunctionType
OP = mybir.AluOpType

LAST_RESULT = None


def _mm(nc, out, lhsT, rhs, **kw):
    nc.tensor.matmul(out, lhsT, rhs, **kw)


def build_nc(reps=1):
    nc = bacc.Bacc()

    qTin = nc.dram_tensor("qTin", [D, LC], MDT, kind="ExternalInput")
    kTin = nc.dram_tensor("kTin", [D, S], MDT, kind="ExternalInput")
    vTin = nc.dram_tensor("vTin", [D, S], MDT, kind="ExternalInput")
    Wq = nc.dram_tensor("Wq", [D, D], MDT, kind="ExternalInput")
    Wk = nc.dram_tensor("Wk", [D, D], MDT, kind="ExternalInput")
    Wv = nc.dram_tensor("Wv", [D, D], MDT, kind="ExternalInput")
    Wo = nc.dram_tensor("Wo", [D, D], MDT, kind="ExternalInput")
    bq = nc.dram_tensor("bq", [D], F32, kind="ExternalInput")
    bo2 = nc.dram_tensor("bo2", [D], F32, kind="ExternalInput")
    tau = nc.dram_tensor("tau", [1], F32, kind="ExternalInput")
    delta = nc.dram_tensor("delta", [S], F32, kind="ExternalInput")
    out = nc.dram_tensor("out", [LC, D], F32, kind="ExternalOutput")

    kTin_r = kTin.rearrange("(j p) s -> p j s", p=128)
    qTin_r = qTin.rearrange("(j p) l -> p j l", p=128)
    vTin_r = vTin.rearrange("(j p) s -> p j s", p=128)

    with ExitStack() as ctx:
        tc = ctx.enter_context(tile.TileContext(nc))
        consts = ctx.enter_context(tc.tile_pool(name="consts", bufs=1))
        proj = ctx.enter_context(tc.tile_pool(name="proj", bufs=1))
        pin = ctx.enter_context(tc.tile_pool(name="pin", bufs=1))
        kqr = ctx.enter_context(tc.tile_pool(name="kqr", bufs=2))
        vsl = ctx.enter_context(tc.tile_pool(name="vsl", bufs=2))
        pp = ctx.enter_context(tc.tile_pool(name="pp", bufs=3))
        onp = ctx.enter_context(tc.tile_pool(name="onp", bufs=8))
        rcb = ctx.enter_context(tc.tile_pool(name="rcb", bufs=3))
        rbp = ctx.enter_context(tc.tile_pool(name="rbp", bufs=3))
        fsp = ctx.enter_context(tc.tile_pool(name="fsp", bufs=2))
        qkp = ctx.enter_context(tc.tile_pool(name="qkp", bufs=2, space="PSUM"))
        avp = ctx.enter_context(tc.tile_pool(name="avp", bufs=2, space="PSUM"))
        pp2 = ctx.enter_context(tc.tile_pool(name="pp2", bufs=2, space="PSUM"))

        # --- small constants -------------------------------------------------
        # sync queue: bq, tau, delta, Wk, kTin0, kTin1
        bq_sb = consts.tile([128, 4], F32)
        nc.sync.dma_start(out=bq_sb, in_=bq.rearrange("(j p) -> p j", p=128))
        tau_bc0 = consts.tile([128, 1], F32)
        nc.sync.dma_start(
            out=tau_bc0,
            in_=tau.rearrange("(a b) -> a b", a=1).to_broadcast([128, 1]))
        tau_bc = consts.tile([128, 1], F32)
        nc.vector.tensor_scalar(out=tau_bc, in0=tau_bc0, scalar1=0.125,
                                scalar2=None, op0=OP.mult)  # tau/sqrt(dk)
        delta_sb = consts.tile([128, SC], F32)
        nc.sync.dma_start(out=delta_sb, in_=delta.rearrange("(j p) -> p j", p=128))
        w_sb = consts.tile([128, SC], F32)  # w[s] = exp(delta[s])
        nc.scalar.activation(w_sb, delta_sb, AF.Exp)

        # big inputs: spread across the three DMA-capable queues (sync/SP,
        # scalar/ACT, gpsimd/SWDGE). Total input DMA is the lead-in
        # bottleneck (HBM-bandwidth serialized), so only what gates the
        # first few phases is issued up front; the rest is emitted at the
        # program point just before its consumer.
        Wv_sb = consts.tile([128, 4, D], MDT)
        nc.scalar.dma_start(out=Wv_sb, in_=Wv.rearrange("(j p) n -> p j n", p=128))
        Wq_sb = consts.tile([128, 4, D], MDT)
        nc.scalar.dma_start(out=Wq_sb, in_=Wq.rearrange("(j p) n -> p j n", p=128))
        Wk_sb = consts.tile([128, 4, D], MDT)
        nc.sync.dma_start(out=Wk_sb, in_=Wk.rearrange("(j p) n -> p j n", p=128))
        kTin_sb = pin.tile([128, 4, S], MDT)
        nc.sync.dma_start(out=kTin_sb[:, :, 0:512], in_=kTin_r[:, :, 0:512])
        qTin_sb = pin.tile([128, 4, LC], MDT)
        nc.sync.dma_start(out=qTin_sb[:, :, 0:512], in_=qTin_r[:, :, 0:512])
        nc.sync.dma_start(out=kTin_sb[:, :, 512:1024],
                          in_=kTin_r[:, :, 512:1024])
        # Wo rows for head pair hp at partitions 0..127 (h even: 0-63, h odd:
        # 64-127) — the output projection contracts stacked head pairs.
        # DMA'd late (emitted at hp==2) — only needed by the output phase.
        Wo_sb = consts.tile([128, 4, D], MDT)
        bo2_bc = consts.tile([128, D], F32)

        # persistent across all phases: weighted values [w*v | w]
        vw_sb = proj.tile([128, SC, H, 65], ADT)

        for _rep in range(reps):
            otp = {}
            vgrp = {}

            def emit_vgrp_dma(g):
                # one SWDGE issue per 4 v chunks (per-chunk issues are ~1.2us
                # of Pool time each and serialize the lead-in)
                vgrp[g] = vsl.tile([128, 4, 512], MDT, name=f"vg_{g}", tag="vg")
                nc.gpsimd.dma_start(out=vgrp[g],
                                    in_=vTin_r[:, :, g * 512:(g + 1) * 512])

            def emit_vproj(st):
                g, o = divmod(st, 4)
                vsl_t = vgrp[g][:, :, o * 128:(o + 1) * 128]
                ps = pp2.tile([128, 512], F32, name=f"psv_{st}", tag="ps")
                for ji in range(4):
                    _mm(nc, ps, vsl_t[:, ji, :], Wv_sb[:, ji, :],
                        start=(ji == 0), stop=(ji == 3))
                nc.vector.tensor_scalar(
                    out=vw_sb[:, st, :, 0:64],
                    in0=ps.rearrange("p (h d) -> p h d", h=H),
                    scalar1=w_sb[:, st:st + 1], scalar2=None, op0=OP.mult)
                nc.vector.tensor_copy(
                    out=vw_sb[:, st, :, 64:65],
                    in_=w_sb[:, st:st + 1].to_broadcast([128, H, 1]))

            kqt = {}

            def make_kq(hp):
                kqt[hp] = (
                    kqr.tile([128, S], ADT, name=f"kT_{hp}", tag="kT"),
                    kqr.tile([128, LC], ADT, name=f"qT_{hp}", tag="qT"),
                )

            def emit_kproj(hp, st):
                ps = pp2.tile([128, 512], F32, name=f"psk_{hp}_{st}", tag="ps")
                for ji in range(4):
                    _mm(nc, ps, Wk_sb[:, ji, hp * 128:(hp + 1) * 128],
                        kTin_sb[:, ji, st * 512:(st + 1) * 512],
                        start=(ji == 0), stop=(ji == 3))
                nc.vector.tensor_copy(
                    out=kqt[hp][0][:, st * 512:(st + 1) * 512], in_=ps)

            def emit_qproj(hp, lt):
                ps = pp2.tile([128, 512], F32, name=f"psq_{hp}_{lt}", tag="ps")
                for ji in range(4):
                    _mm(nc, ps, Wq_sb[:, ji, hp * 128:(hp + 1) * 128],
                        qTin_sb[:, ji, lt * 512:(lt + 1) * 512],
                        start=(ji == 0), stop=(ji == 3))
                nc.vector.tensor_scalar(
                    out=kqt[hp][1][:, lt * 512:(lt + 1) * 512], in0=ps,
                    scalar1=bq_sb[:, hp:hp + 1], scalar2=None, op0=OP.add)

            def emit_oproj(qt, i):
                # output projection for q rows [qt*512 + i*128, +128): stacked
                # head pairs contract over K=128 (h even dims 0-63, h odd
                # dims 64-127), matching Wo_sb's (j p) row packing.
                # fps lives in the proj-psum pool: projections are done by the
                # time the output phase runs, so they never contend, and this
                # keeps the av pair plus both fps buffers within 8 banks.
                fps = pp2.tile([128, 512], F32, name=f"fps_{qt}_{i}", tag="ps")
                for hp in range(H // 2):
                    _mm(nc, fps, otp[(qt, hp)][:, i * 128:(i + 1) * 128],
                        Wo_sb[:, hp, :], start=(hp == 0), stop=(hp == H // 2 - 1))
                fsb = fsp.tile([128, 512], F32, name=f"fsb_{qt}_{i}", tag="fsb")
                nc.vector.tensor_add(fsb, fps, bo2_bc)
                r0 = qt * 512 + i * 128
                nc.sync.dma_start(out=out[r0:r0 + 128, :], in_=fsb)

            # lead-in: first v chunks + head-pair 0 projections. ACT idles
            # here waiting on input DMA, so front-load six vproj groups to
            # keep the attention s-loop slots light.
            emit_vgrp_dma(0)
            emit_vgrp_dma(1)
            for st in range(4):
                emit_vproj(st)
            make_kq(0)
            emit_kproj(0, 0)
            emit_qproj(0, 0)

            for hp in range(H // 2):
                h0, h1 = 2 * hp, 2 * hp + 1
                kT_sb, qT_sb = kqt[hp]

                for qt in range(NQT):
                    # work interleaved into this (hp, qt) s-loop, keyed by scp
                    extras = {}
                    if hp == 0 and qt == 0:
                        def _ktin2_dma():
                            nc.sync.dma_start(out=kTin_sb[:, :, 1024:1536],
                                              in_=kTin_r[:, :, 1024:1536])

                        def _ktin3_dma():
                            nc.sync.dma_start(out=kTin_sb[:, :, 1536:2048],
                                              in_=kTin_r[:, :, 1536:2048])

                        def _qtin1_dma():
                            nc.scalar.dma_start(out=qTin_sb[:, :, 512:1024],
                                                in_=qTin_r[:, :, 512:1024])

                        extras = {
                            0: [lambda: emit_vproj(4), lambda: emit_vproj(5),
                                lambda: emit_vgrp_dma(2), _ktin2_dma,
                                lambda: emit_kproj(0, 1)],
                            1: [lambda: emit_vproj(6), lambda: emit_vproj(7),
                                lambda: emit_kproj(0, 2)],
                            2: [lambda: emit_vproj(8), lambda: emit_vproj(9),
                                lambda: emit_vgrp_dma(3), _ktin3_dma,
                                _qtin1_dma, lambda: emit_kproj(0, 3)],
                            3: [lambda: emit_vproj(10), lambda: emit_vproj(11)],
                            4: [lambda: emit_vproj(12), lambda: emit_vproj(13),
                                lambda: emit_qproj(0, 1)],
                            5: [lambda: emit_vproj(14), lambda: emit_vproj(15)],
                        }
                    elif qt == 1 and hp < H // 2 - 1:
                        hn = hp + 1

                        def _wo_dma():
                            nc.scalar.dma_start(
                                out=Wo_sb,
                                in_=Wo.rearrange("(j p) n -> p j n", p=128))
                            nc.scalar.dma_start(
                                out=bo2_bc,
                                in_=bo2.rearrange("(a n) -> a n", a=1)
                                .to_broadcast([128, D]))

                        extras = {
                            0: [lambda: make_kq(hn), lambda: emit_kproj(hn, 0)],
                            1: [lambda: emit_kproj(hn, 1)],
                            2: [lambda: emit_kproj(hn, 2)],
                            3: [lambda: emit_kproj(hn, 3)],
                            4: [lambda: emit_qproj(hn, 0)],
                            5: [lambda: emit_qproj(hn, 1)],
                        }
                        if hp == 1:
                            extras[6] = [_wo_dma]
                    elif qt == 1 and hp == H // 2 - 1:
                        extras = {
                            1: [lambda: emit_oproj(0, 0)],
                            3: [lambda: emit_oproj(0, 1)],
                            5: [lambda: emit_oproj(0, 2)],
                            7: [lambda: emit_oproj(0, 3)],
                        }

                    av = [avp.tile([128, 512], F32, name=f"av_{qt}_{hp}_{j}",
                                   tag="avf") for j in range(2)]
                    for scp in range(SC // 2):
                        for th in extras.get(scp, []):
                            th()
                        qk0 = qkp.tile([128, 1024], F32,
                                       name=f"qk0_{qt}_{hp}_{scp}", tag="qk")
                        qk1 = qkp.tile([128, 1024], F32,
                                       name=f"qk1_{qt}_{hp}_{scp}", tag="qk")
                        for k2 in range(2):
                            sc = 2 * scp + k2
                            # heads of the pair live on partition halves of the
                            # kT/qT pass tiles -> concurrent row-tiled matmuls
                            _mm(nc, qk0[:, k2 * 512:(k2 + 1) * 512],
                                kT_sb[0:64, sc * 128:(sc + 1) * 128],
                                qT_sb[0:64, qt * 512:(qt + 1) * 512],
                                start=True, stop=True)
                            _mm(nc, qk1[:, k2 * 512:(k2 + 1) * 512],
                                kT_sb[64:128, sc * 128:(sc + 1) * 128],
                                qT_sb[64:128, qt * 512:(qt + 1) * 512],
                                start=True, stop=True)
                        p0 = pp.tile([128, 1024], ADT,
                                     name=f"p0_{qt}_{hp}_{scp}", tag="p")
                        p1 = pp.tile([128, 1024], ADT,
                                     name=f"p1_{qt}_{hp}_{scp}", tag="p")
                        nc.scalar.activation(p0, qk0, AF.Exp, scale=tau_bc)
                        nc.scalar.activation(p1, qk1, AF.Exp, scale=tau_bc)
                        for k2 in range(2):
                            sc = 2 * scp + k2
                            _mm(nc, av[0][0:65, :], vw_sb[:, sc, h0, :],
                                p0[:, k2 * 512:(k2 + 1) * 512],
                                start=(sc == 0), stop=(sc == SC - 1))
                            _mm(nc, av[1][0:65, :], vw_sb[:, sc, h1, :],
                                p1[:, k2 * 512:(k2 + 1) * 512],
                                start=(sc == 0), stop=(sc == SC - 1))

                    # softmax normalize: reciprocal of the denominator row,
                    # partition-broadcast (Pool engine), multiply. Head pair
                    # results stack into one [128, 512] tile for the stacked
                    # output projection.
                    ott = onp.tile([128, 512], MDT, name=f"ot_{qt}_{hp}",
                                   tag="ot")
                    otp[(qt, hp)] = ott
                    for i2 in range(2):
                        rcp_r = rcb.tile([1, 512], F32, name=f"rc_{qt}_{hp}_{i2}",
                                         tag="rc")
                        nc.vector.reciprocal(rcp_r, av[i2][64:65, :])
                        rb = rbp.tile([64, 512], F32, name=f"rb_{qt}_{hp}_{i2}",
                                      tag="rb")
                        nc.gpsimd.partition_broadcast(rb, rcp_r)
                        nc.vector.tensor_mul(ott[i2 * 64:(i2 + 1) * 64, :],
                                             av[i2][0:64, :], rb)

                    if hp == H // 2 - 1 and qt == NQT - 1:
                        for i in range(4):
                            emit_oproj(1, i)

    return nc


_NC_CACHE = None


def _get_nc():
    global _NC_CACHE
    if _NC_CACHE is None:
        _NC_CACHE = build_nc()
        _NC_CACHE.finalize()
    return _NC_CACHE


def prep_in_maps(queries, keys, values, tau, delta, Wq, bq, Wk, bk, Wv, bv,
                 Wo, bo, **_unused):
    queries = np.asarray(queries, NPDT)
    keys = np.asarray(keys, NPDT)
    values = np.asarray(values, NPDT)
    tau = np.asarray(tau, np.float32)
    delta = np.ascontiguousarray(np.asarray(delta, np.float32))
    # bo2 = bv @ Wo + bo (exact: attention rows sum to 1). bk is dropped:
    # it shifts every score of a query row equally, which softmax cancels.
    bo2 = (np.asarray(bv, np.float64) @ np.asarray(Wo, np.float64)
           + np.asarray(bo, np.float64)).astype(np.float32)
    shared = {
        "Wq": np.ascontiguousarray(np.asarray(Wq, NPDT)),
        "Wk": np.ascontiguousarray(np.asarray(Wk, NPDT)),
        "Wv": np.ascontiguousarray(np.asarray(Wv, NPDT)),
        "Wo": np.ascontiguousarray(np.asarray(Wo, NPDT)),
        "bq": np.ascontiguousarray(np.asarray(bq, np.float32)),
        "bo2": np.ascontiguousarray(bo2),
    }

    in_maps = []
    for c in range(NCORES):
        b, hf = divmod(c, 2)
        in_maps.append({
            "qTin": np.ascontiguousarray(
                queries[b, hf * LC:(hf + 1) * LC, :].T),
            "kTin": np.ascontiguousarray(keys[b].T),
            "vTin": np.ascontiguousarray(values[b].T),
            "tau": np.ascontiguousarray(tau[b:b + 1]),
            "delta": np.ascontiguousarray(delta[b]),
            **shared,
        })
    return in_maps


def kernel(**inputs):
    in_maps = prep_in_maps(**inputs)
    nc = _get_nc()
    res = run_bass_kernel_spmd(
        nc, in_maps, core_ids=list(range(NCORES)),
        trace=os.environ.get("KERNEL_TRACE") == "1")
    global LAST_RESULT
    LAST_RESULT = res

    out = np.empty((B, LFULL, D), np.float32)
    for c in range(NCORES):
        b, hf = divmod(c, 2)
        out[b, hf * LC:(hf + 1) * LC, :] = res.results[c]["out"]
    return out
